# revision 29
# baseline (speedup 1.0000x reference)
"""Trainium2 Bass kernel for BiasedMHA (B=4, N=1024, C=1024, H=16, D=64).

Sharding: 8 cores = 4 batches x 2 head-halves (8 heads each).
Each core computes q/k/v projections for its head slice, biased softmax
attention, and its partial o_proj.  Host sums the two partials per batch
and adds the (bo + bv @ Wo.T) constant.

v8: PE-density-first + latency-loop-free pipeline.  The PE p-state ramp
demotes to ~2x cycle time after ANY idle and needs >3us of continuous
execution to recover, so the whole kernel keeps the PE the strict
bottleneck with deep enough buffering that no WAR/RAW edge can stall
the in-order PE stream:
- attention processes heads SINGLY (not in pairs): the PV accumulator
  then needs only one [128,1024] PSUM tile (2 banks), freeing budget
  for THREE S-psum tiles (6 banks).  With a 3-deep S rotation and
  lag-2 PV emission every cross-engine dependency is >= 2 steps old.
- attn_bias lands in PSUM via identity matmul (start=True): useful PE
  filler; exp reads S+B straight from PSUM.
- wide denominator: the PV stationary carries 64 ones-columns, so PSUM
  rows 64:127 hold the softmax denominator replicated.  Normalize is
  den-copy/body-copy/recip/mult on DVE only.
- projections pipeline 4-deep across both PSUM pools; evacuations
  alternate DVE / ACT-Copy; DMA arrival order matches consumption.

Layouts (host-prepped, contraction-on-partitions):
  xT    [C, N]  fp16 : x[b].T
  wqT   [C, 512] fp16: Wq[rows,:].T * scale (softmax scale folded)
  wkT/wvT [C, 512] fp16
  woT   [512, C] fp16: Wo[:, cols].T
  biasT [8, N, N] fp16: attn_bias[b, local head][query, key].T
  bqr/bkr [1, 512] fp16: bias rows via K=1 matmuls (skipped when the
        host detects all-zero q/k biases)
  ident [128, 128] fp16: identity (stationary of the bias-add matmul)
  madd  [128, NT] fp32: additive mask (-1e30 where attn_mask==0)
"""

import sys

if "/opt/trn_rl_repo" not in sys.path:
    sys.path.insert(0, "/opt/trn_rl_repo")

from contextlib import ExitStack

import numpy as np

B, N, C, H = 4, 1024, 1024, 16
D = C // H            # 64
HL = H // 2           # 8 local heads per core
JL = HL * D           # 512 local head dims
NT = N // 128         # 8 seq tiles
CT = C // 128         # 8 contraction tiles
SCALE = D ** (-0.5)

_prog_cache = {}


def build_program(with_qk_bias=True):
    import concourse.tile as tile
    from concourse import bacc, mybir
    f32 = mybir.dt.float32
    f16 = mybir.dt.float16

    nc = bacc.Bacc("TRN2", target_bir_lowering=False, debug=False,
                   enable_asserts=False, num_devices=8)

    xT = nc.dram_tensor("xT", [C, N], f16, kind="ExternalInput").ap()
    wqT = nc.dram_tensor("wqT", [C, JL], f16, kind="ExternalInput").ap()
    wkT = nc.dram_tensor("wkT", [C, JL], f16, kind="ExternalInput").ap()
    wvT = nc.dram_tensor("wvT", [C, JL], f16, kind="ExternalInput").ap()
    woT = nc.dram_tensor("woT", [JL, C], f16, kind="ExternalInput").ap()
    bqr = nc.dram_tensor("bqr", [1, JL], f16, kind="ExternalInput").ap()
    bkr = nc.dram_tensor("bkr", [1, JL], f16, kind="ExternalInput").ap()
    ident = nc.dram_tensor("ident", [128, 128], f16, kind="ExternalInput").ap()
    biasT = nc.dram_tensor("biasT", [HL, N, N], f16,
                           kind="ExternalInput").ap()
    madd = nc.dram_tensor("madd", [128, NT], f32, kind="ExternalInput").ap()
    outp = nc.dram_tensor("outp", [N, C], f16, kind="ExternalOutput").ap()

    Exp = mybir.ActivationFunctionType.Exp
    Copy = mybir.ActivationFunctionType.Copy
    mult_op = mybir.AluOpType.mult
    from concourse.tile_rust import add_dep_helper
    first_evac = [None]

    with tile.TileContext(nc) as tc, ExitStack() as ctx:
        # ---- pools ----
        resident = ctx.enter_context(tc.tile_pool(name="resident", bufs=1))
        biaspool = ctx.enter_context(tc.tile_pool(name="bias", bufs=4))
        exppool = ctx.enter_context(tc.tile_pool(name="exps", bufs=5))
        outpool = ctx.enter_context(tc.tile_pool(name="outs", bufs=3))
        smallpool = ctx.enter_context(tc.tile_pool(name="small", bufs=4))
        ps_main = ctx.enter_context(
            tc.tile_pool(name="ps_main", bufs=2, space="PSUM"))
        ps_pv = ctx.enter_context(
            tc.tile_pool(name="ps_pv", bufs=2, space="PSUM"))

        # ---- resident tiles ----
        # chunked + consumption-ordered loads: DMA queues are global FIFOs,
        # so enqueue bytes in the order the projections will need them
        xts = resident.tile([128, CT, N], f16)          # xT tiled on c
        wq_sb = resident.tile([128, CT, JL], f16)       # wqT tiled on c
        wk_sb = resident.tile([128, CT, JL], f16)       # wkT tiled on c
        wv_sb = resident.tile([128, CT, JL], f16)       # wvT tiled on c
        wo_sb = resident.tile([128, 4, C], f16)         # woT tiled on j
        xview = xT.rearrange("(ct p) n -> p ct n", p=128)
        wqview = wqT.rearrange("(ct p) j -> p ct j", p=128)
        wkview = wkT.rearrange("(ct p) j -> p ct j", p=128)
        d0 = None
        for ct in range(CT):
            nc.sync.dma_start(wq_sb[:, ct, 0:256], wqview[:, ct, 0:256])
            last = nc.sync.dma_start(xts[:, ct, :], xview[:, ct, :])
            if d0 is None:
                d0 = last
            # wk jt0-half rides along per-ct so k-jp0 never waits on bulk
            nc.sync.dma_start(wk_sb[:, ct, 0:256], wkview[:, ct, 0:256])
        # consumption order: q-jp1, k-jp1, v
        bulk = [
            nc.sync.dma_start(wq_sb[:, :, 256:512], wqview[:, :, 256:512]),
            nc.sync.dma_start(wk_sb[:, :, 256:512], wkview[:, :, 256:512]),
            nc.sync.dma_start(wv_sb[:],
                              wvT.rearrange("(ct p) j -> p ct j", p=128)),
            nc.sync.dma_start(wo_sb[:],
                              woT.rearrange("(kt p) c -> p kt c", p=128)),
        ]
        for b in bulk:
            add_dep_helper(b.ins, d0.ins, reason="startup DMA priority")

        id_sb = resident.tile([128, 128], f16)
        nc.sync.dma_start(id_sb[:], ident)

        madd_sb = resident.tile([128, NT], f32)
        nc.sync.dma_start(madd_sb[:], madd)

        bq_sb = resident.tile([1, JL], f16)
        nc.sync.dma_start(bq_sb[:], bqr)
        bk_sb = resident.tile([1, JL], f16)
        nc.sync.dma_start(bk_sb[:], bkr)

        qT_sb = resident.tile([128, 4, N], f16)         # [j-tile, n]
        # k stored zero-padded per head: rows 0:64 = kT_h, rows 64:128 = 0,
        # so the S stationary is always a full (128,128) tile -- the PE
        # charges ~105ns to reconfigure whenever the stationary tile size
        # changes, and this keeps every matmul in (128,128) mode.
        kZ_sb = resident.tile([128, HL, N], f16)
        # v with 64 ones-columns per head: [m-tile, h*(64 V + 64 ones)]
        v_sb = resident.tile([128, NT, HL * 2 * D], f16)
        # attT as one tile per head-pair (o_proj stationary layout)
        attT_t = [resident.tile([128, N], f16, name=f"attT_{i}")
                  for i in range(4)]

        ones_f32 = resident.tile([128, 1], f32)
        nc.vector.memset(ones_f32[:], 1.0)
        ones_row = None
        if with_qk_bias:
            ones_row = resident.tile([1, N], f16)
            nc.vector.tensor_copy(
                ones_row[:], ones_f32[0:1, 0:1].to_broadcast([1, N]))

        # even heads: kT in rows 0:64 (q rows 0:64), zeros below;
        # odd heads: kT in rows 64:128 (q rows 64:128), zeros above
        nc.gpsimd.memset(kZ_sb[64:128, 0:HL:2, :], 0.0)
        nc.gpsimd.memset(kZ_sb[0:64, 1:HL:2, :], 0.0)

        # ones columns of v (wide softmax-denominator trick)
        for mt in range(NT):
            v4o = v_sb[:, mt, :].rearrange("p (h c) -> p h c", c=2 * D)
            nc.vector.tensor_copy(
                v4o[:, :, D:2 * D],
                ones_f32[:, 0:1, None].to_broadcast([128, HL, D]))



        # ---- PE warm-up while the first DMAs land ----
        warm_sb = resident.tile([128, 512], f16)
        nc.vector.memset(warm_sb[:], 0.0)
        warm_ps = ps_pv.tile([128, 512], f32, tag="pv", name="warm")
        for i in range(4):
            nc.tensor.matmul(warm_ps[:], warm_sb[:, 0:128], warm_sb[:],
                             start=True, stop=True)

        # ---- phase A: projections, 4-deep across both PSUM pools ----
        evac_ctr = [0]

        def evac(dst, src):
            # alternate DVE / ACT so neither engine gates the projections
            ev = evac_ctr[0]
            evac_ctr[0] += 1
            if ev % 2 == 0:
                return nc.vector.tensor_copy(dst, src)
            return nc.scalar.activation(dst, src, Copy)

        def emit_qk_group(wsb, brow, dest, jp):
            ps2 = [ps_main.tile([128, N], f32, tag="mm", name=f"proj_{jp}_0"),
                   ps_pv.tile([128, N], f32, tag="pv", name=f"proj_{jp}_1")]
            for ct in range(CT):
                for ji in range(2):
                    jt = jp * 2 + ji
                    w = wsb[:, ct, jt * 128:(jt + 1) * 128]
                    for nh in range(2):
                        nc.tensor.matmul(
                            ps2[ji][:, nh * 512:(nh + 1) * 512],
                            w[:],
                            xts[:, ct, nh * 512:(nh + 1) * 512],
                            start=(ct == 0),
                            stop=(not with_qk_bias and ct == CT - 1))
            for ji in range(2):
                jt = jp * 2 + ji
                if with_qk_bias:
                    for nh in range(2):
                        nc.tensor.matmul(
                            ps2[ji][:, nh * 512:(nh + 1) * 512],
                            brow[0:1, jt * 128:(jt + 1) * 128],
                            ones_row[0:1, nh * 512:(nh + 1) * 512],
                            start=False, stop=True)
                ev = evac(dest[:, jt, :], ps2[ji][:])
                if first_evac[0] is None:
                    first_evac[0] = ev

        def emit_v_group(mt, pool, tag):
            # v normal layout: out[m-tile, j] = sum_c xT[c, m] * wvT[c, j]
            ps = pool.tile([128, N], f32, tag=tag, name=f"vproj_{mt}")
            psv = ps[:, 0:JL]
            for ct in range(CT):
                nc.tensor.matmul(
                    psv,
                    xts[:, ct, mt * 128:(mt + 1) * 128],
                    wv_sb[:, ct, :],
                    start=(ct == 0), stop=(ct == CT - 1))
            v4 = v_sb[:, mt, :].rearrange("p (h c) -> p h c", c=2 * D)
            evac(v4[:, :, 0:D], psv.rearrange("p (h c) -> p h c", c=D))

        def emit_k_group(jp):
            ps2 = [ps_main.tile([128, N], f32, tag="mm", name=f"kproj_{jp}_0"),
                   ps_pv.tile([128, N], f32, tag="pv", name=f"kproj_{jp}_1")]
            for ct in range(CT):
                for ji in range(2):
                    jt = jp * 2 + ji
                    w = wk_sb[:, ct, jt * 128:(jt + 1) * 128]
                    for nh in range(2):
                        nc.tensor.matmul(
                            ps2[ji][:, nh * 512:(nh + 1) * 512],
                            w[:],
                            xts[:, ct, nh * 512:(nh + 1) * 512],
                            start=(ct == 0),
                            stop=(not with_qk_bias and ct == CT - 1))
            for ji in range(2):
                jt = jp * 2 + ji
                if with_qk_bias:
                    for nh in range(2):
                        nc.tensor.matmul(
                            ps2[ji][:, nh * 512:(nh + 1) * 512],
                            bk_sb[0:1, jt * 128:(jt + 1) * 128],
                            ones_row[0:1, nh * 512:(nh + 1) * 512],
                            start=False, stop=True)
                evac(kZ_sb[0:64, 2 * jt, :], ps2[ji][0:64, :])
                evac(kZ_sb[64:128, 2 * jt + 1, :], ps2[ji][64:128, :])

        emit_qk_group(wq_sb, bq_sb, qT_sb, 0)
        emit_k_group(0)
        emit_qk_group(wq_sb, bq_sb, qT_sb, 1)
        emit_k_group(1)
        for mt in range(NT):
            pool, tag = ((ps_pv, "pv") if mt % 2 else (ps_main, "mm"))
            emit_v_group(mt, pool, tag)

        # ---- phase B: attention, one head at a time ----
        # 3-deep S-psum rotation + lag-2 PV: every cross-engine edge the
        # in-order PE stream waits on is >= 2 steps old, so the PE never
        # idles (which would demote the p-state).
        bias_view = biasT.rearrange("h (t p) n -> h p t n", p=128)

        def emit_scores(h, mt, eb, par):
            sp = ps_main.tile([128, N], f32, tag="mm", name=f"s_{h}_{mt}")
            jt = h // 2
            # keep-warm filler: with all matmuls in (128,128) mode the PE
            # per-step cost (1065ns) would dip under ACT's exp (1095ns) and
            # idle slivers would demote the p-state; a 256-col dummy into
            # the about-to-be-reset psum keeps the PE strictly bottleneck
            # bias folded entirely as exp(S)*exp(B) on DVE: the lag-3
            # pipeline is ACT-bound, so PE work is minimized
            for nh in range(2):
                sl = slice(nh * 512, (nh + 1) * 512)
                nc.tensor.matmul(
                    sp[:, sl],
                    kZ_sb[:, h, mt * 128:(mt + 1) * 128],
                    qT_sb[:, jt, sl],
                    start=True, stop=True)
            et = exppool.tile([128, N], f16, tag="exp", name=f"exp_{h}_{mt}")
            nc.scalar.activation(et[:], sp[:], Exp,
                                 bias=madd_sb[:, mt:mt + 1])
            nc.vector.tensor_tensor(et[:], et[:], eb[:, par, :], mult_op)
            return et

        def emit_pv(h, mt, pv, et):
            vx = v_sb[:, mt, h * 128:(h + 1) * 128]
            for nh in range(2):
                sl = slice(nh * 512, (nh + 1) * 512)
                nc.tensor.matmul(
                    pv[:, sl], vx, et[:, sl],
                    start=(mt == 0), stop=(mt == NT - 1))

        def emit_normalize(h, pv):
            # rows 64:127 of pv hold the denominator replicated; DVE-only
            # chain so the PE stream has nothing to stall on here.  For the
            # last head this chain is the kernel's tail critical path: the
            # body copy moves to the (by then idle) ACT engine and the
            # final multiply is split so o_proj's first kt=3 matmuls can
            # start after the first half.
            last = (h == HL - 1)
            dst = attT_t[h // 2][(h % 2) * 64:(h % 2) * 64 + 64, :]
            if last:
                # per-half chain so o_proj's kt=3 matmuls for the first
                # n-tiles can start after ~half the normalize latency
                for nh in range(2):
                    sl = slice(nh * 512, (nh + 1) * 512)
                    d_ = smallpool.tile([64, 512], f32, tag="den",
                                        name=f"den_{h}_{nh}")
                    nc.vector.tensor_copy(d_[:], pv[64:128, sl])
                    r = smallpool.tile([64, 512], f32, tag="recip",
                                       name=f"recip_{h}_{nh}")
                    nc.vector.reciprocal_approx_fast(out=r[:], in_=d_[:])
                    nc.vector.tensor_tensor(dst[:, sl], pv[0:64, sl],
                                            r[:], mult_op)
            else:
                d_ = smallpool.tile([64, N], f32, tag="den", name=f"den_{h}")
                nc.vector.tensor_copy(d_[:], pv[64:128, :])
                r = smallpool.tile([64, N], f32, tag="recip",
                                   name=f"recip_{h}")
                nc.vector.reciprocal_approx_fast(out=r[:], in_=d_[:])
                # multiply straight off the PV PSUM (no body copy)
                nc.vector.tensor_tensor(dst, pv[0:64, :], r[:], mult_op)

        pv_by_head = {}
        eb_cur = [None]
        pending = []
        for g in range(HL * NT):
            h, mt = divmod(g, NT)
            if mt == 0:
                pv_by_head[h] = ps_pv.tile([128, N], f32, tag="pv",
                                           name=f"pv_{h}")
            if mt % 4 == 0:
                eb = biaspool.tile([128, 4, N], f16, tag="bias",
                                   name=f"bias_{h}_{mt}")
                bdma = nc.gpsimd.dma_start(
                    eb[:], bias_view[h, :, mt:mt + 4, :])
                if h == 0 and first_evac[0] is not None:
                    add_dep_helper(bdma.ins, first_evac[0].ins,
                                   reason="bias prefetch behind startup")
                eb_cur[0] = eb
            et = emit_scores(h, mt, eb_cur[0], mt % 4)
            pending.append((h, mt, et))
            if g >= 3:
                ph, pmt, pet = pending.pop(0)
                emit_pv(ph, pmt, pv_by_head[ph], pet)
                if pmt == NT - 1:
                    emit_normalize(ph, pv_by_head[ph])
        while pending:
            ph, pmt, pet = pending.pop(0)
            emit_pv(ph, pmt, pv_by_head[ph], pet)
            if pmt == NT - 1:
                emit_normalize(ph, pv_by_head[ph])

        # ---- phase C: o_proj partial ----
        # kt-outer so both ch matmuls share the same stationary attT slice.
        # Three groups open with kt=0..2 before any kt=3, so the PE stream
        # has ~4us of ready work while the last pair's normalize (DVE)
        # finishes -- no idle, no p-state demotion.
        def open_group(nt):
            pool, tag = ((ps_pv, "pv") if nt % 2 else (ps_main, "mm"))
            ps = pool.tile([128, N], f32, tag=tag, name=f"oproj_{nt}")
            for kt in range(3):
                att = attT_t[kt][:, nt * 128:(nt + 1) * 128]
                for ch in range(2):
                    nc.tensor.matmul(
                        ps[:, ch * 512:(ch + 1) * 512],
                        att,
                        wo_sb[:, kt, ch * 512:(ch + 1) * 512],
                        start=(kt == 0), stop=False)
            return ps

        def close_group(nt, ps):
            att = attT_t[3][:, nt * 128:(nt + 1) * 128]
            for ch in range(2):
                nc.tensor.matmul(
                    ps[:, ch * 512:(ch + 1) * 512],
                    att,
                    wo_sb[:, 3, ch * 512:(ch + 1) * 512],
                    start=False, stop=True)
            ot = outpool.tile([128, N], f16, tag="out")
            if nt == NT - 1:
                # split the last tile's evac+store so the DMA of the first
                # half overlaps the evacuation of the second (shorter tail)
                for h2 in range(2):
                    sl = slice(h2 * 512, (h2 + 1) * 512)
                    (nc.scalar.activation(ot[:, sl], ps[:, sl], Copy)
                     if h2 == 0 else
                     nc.vector.tensor_copy(ot[:, sl], ps[:, sl]))
                    nc.sync.dma_start(outp[nt * 128:(nt + 1) * 128, sl],
                                      ot[:, sl])
            else:
                if nt % 2 == 0:
                    nc.scalar.activation(ot[:], ps[:], Copy)
                else:
                    nc.vector.tensor_copy(ot[:], ps[:])
                nc.sync.dma_start(outp[nt * 128:(nt + 1) * 128, :], ot[:])

        open_ps = [open_group(0), open_group(1)]
        for nt in range(NT):
            close_group(nt, open_ps[nt])
            if nt + 2 < NT:
                open_ps.append(open_group(nt + 2))

    nc.compile()
    return nc


def get_program(with_qk_bias=True):
    key = ("nc", with_qk_bias)
    if key not in _prog_cache:
        _prog_cache[key] = build_program(with_qk_bias=with_qk_bias)
    return _prog_cache[key]


def make_in_maps(x, attn_bias, attn_mask, Wq, bq, Wk, bk, Wv, bv, Wo, bo):
    """Host-side shard + layout prep.  Returns (in_maps, const) where
    const[c_out] = bo + bv @ Wo.T must be added to the gathered output."""
    x = np.asarray(x, np.float32)
    attn_bias = np.asarray(attn_bias, np.float32)
    attn_mask = np.asarray(attn_mask)
    Wq = np.asarray(Wq, np.float32)
    Wk = np.asarray(Wk, np.float32)
    Wv = np.asarray(Wv, np.float32)
    Wo = np.asarray(Wo, np.float32)
    bq = np.asarray(bq, np.float32)
    bk = np.asarray(bk, np.float32)
    bv = np.asarray(bv, np.float32)
    bo = np.asarray(bo, np.float32)

    const = bo + bv @ Wo.T
    ident = np.eye(128, dtype=np.float16)

    xTs = [np.ascontiguousarray(x[b].T).astype(np.float16) for b in range(B)]
    madds = []
    for b in range(B):
        ma = np.where(attn_mask[b] == 0, np.float32(-1e30), np.float32(0.0))
        madds.append(np.ascontiguousarray(ma.reshape(NT, 128).T))

    in_maps = []
    for core in range(8):
        b, half = divmod(core, 2)
        rows = slice(half * JL, (half + 1) * JL)
        wqT = np.ascontiguousarray(
            (Wq[rows, :] * np.float32(SCALE)).T).astype(np.float16)
        wkT = np.ascontiguousarray(Wk[rows, :].T).astype(np.float16)
        wvT = np.ascontiguousarray(Wv[rows, :].T).astype(np.float16)
        woT = np.ascontiguousarray(Wo[:, rows].T).astype(np.float16)
        bqr = (bq[rows] * np.float32(SCALE)).reshape(1, JL).astype(np.float16)
        bkr = bk[rows].reshape(1, JL).astype(np.float16)
        bT32 = np.ascontiguousarray(
            attn_bias[b, half * HL:(half + 1) * HL].transpose(0, 2, 1))
        # exponentiated: folded as exp(S)*exp(B) on DVE
        bT = np.exp(bT32).astype(np.float16)
        in_maps.append({
            "xT": xTs[b], "wqT": wqT, "wkT": wkT, "wvT": wvT, "woT": woT,
            "bqr": bqr, "bkr": bkr, "ident": ident, "biasT": bT,
            "madd": madds[b],
        })
    return in_maps, const


def gather(results, const):
    out = np.empty((B, N, C), np.float32)
    for b in range(B):
        out[b] = results[2 * b]["outp"].astype(np.float32) \
            + results[2 * b + 1]["outp"].astype(np.float32) \
            + const[None, :]
    return out


def kernel(**inputs):
    from concourse.bass_utils import run_bass_kernel_spmd
    wb = bool(np.any(np.asarray(inputs["bq"]))
              or np.any(np.asarray(inputs["bk"])))
    nc = get_program(with_qk_bias=wb)
    in_maps, const = make_in_maps(**inputs)
    res = run_bass_kernel_spmd(nc, in_maps, core_ids=list(range(8)))
    return gather(res.results, const)


# revision 30
# speedup vs baseline: 1.0252x; 1.0252x over previous
"""Trainium2 Bass kernel for BiasedMHA (B=4, N=1024, C=1024, H=16, D=64).

Sharding: 8 cores = 4 batches x 2 head-halves (8 heads each).
Each core computes q/k/v projections for its head slice, biased softmax
attention, and its partial o_proj.  Host sums the two partials per batch
and adds the (bo + bv @ Wo.T) constant.

v8: PE-density-first + latency-loop-free pipeline.  The PE p-state ramp
demotes to ~2x cycle time after ANY idle and needs >3us of continuous
execution to recover, so the whole kernel keeps the PE the strict
bottleneck with deep enough buffering that no WAR/RAW edge can stall
the in-order PE stream:
- attention processes heads SINGLY (not in pairs): the PV accumulator
  then needs only one [128,1024] PSUM tile (2 banks), freeing budget
  for THREE S-psum tiles (6 banks).  With a 3-deep S rotation and
  lag-2 PV emission every cross-engine dependency is >= 2 steps old.
- attn_bias lands in PSUM via identity matmul (start=True): useful PE
  filler; exp reads S+B straight from PSUM.
- wide denominator: the PV stationary carries 64 ones-columns, so PSUM
  rows 64:127 hold the softmax denominator replicated.  Normalize is
  den-copy/body-copy/recip/mult on DVE only.
- projections pipeline 4-deep across both PSUM pools; evacuations
  alternate DVE / ACT-Copy; DMA arrival order matches consumption.

Layouts (host-prepped, contraction-on-partitions):
  xT    [C, N]  fp16 : x[b].T
  wqT   [C, 512] fp16: Wq[rows,:].T * scale (softmax scale folded)
  wkT/wvT [C, 512] fp16
  woT   [512, C] fp16: Wo[:, cols].T
  biasT [8, N, N] fp16: attn_bias[b, local head][query, key].T
  bqr/bkr [1, 512] fp16: bias rows via K=1 matmuls (skipped when the
        host detects all-zero q/k biases)
  ident [128, 128] fp16: identity (stationary of the bias-add matmul)
  madd  [128, NT] fp32: additive mask (-1e30 where attn_mask==0)
"""

import sys

if "/opt/trn_rl_repo" not in sys.path:
    sys.path.insert(0, "/opt/trn_rl_repo")

from contextlib import ExitStack

import numpy as np

B, N, C, H = 4, 1024, 1024, 16
D = C // H            # 64
HL = H // 2           # 8 local heads per core
JL = HL * D           # 512 local head dims
NT = N // 128         # 8 seq tiles
CT = C // 128         # 8 contraction tiles
SCALE = D ** (-0.5)

_prog_cache = {}


def build_program(with_qk_bias=True):
    import concourse.tile as tile
    from concourse import bacc, mybir
    f32 = mybir.dt.float32
    f16 = mybir.dt.float16

    nc = bacc.Bacc("TRN2", target_bir_lowering=False, debug=False,
                   enable_asserts=False, num_devices=8)

    xT = nc.dram_tensor("xT", [C, N], f16, kind="ExternalInput").ap()
    wqT = nc.dram_tensor("wqT", [C, JL], f16, kind="ExternalInput").ap()
    wkT = nc.dram_tensor("wkT", [C, JL], f16, kind="ExternalInput").ap()
    wvT = nc.dram_tensor("wvT", [C, JL], f16, kind="ExternalInput").ap()
    woT = nc.dram_tensor("woT", [JL, C], f16, kind="ExternalInput").ap()
    bqr = nc.dram_tensor("bqr", [1, JL], f16, kind="ExternalInput").ap()
    bkr = nc.dram_tensor("bkr", [1, JL], f16, kind="ExternalInput").ap()
    ident = nc.dram_tensor("ident", [128, 128], f16, kind="ExternalInput").ap()
    biasT = nc.dram_tensor("biasT", [HL, N, N], f16,
                           kind="ExternalInput").ap()
    madd = nc.dram_tensor("madd", [128, NT], f32, kind="ExternalInput").ap()
    outp = nc.dram_tensor("outp", [N, C], f16, kind="ExternalOutput").ap()

    Exp = mybir.ActivationFunctionType.Exp
    Copy = mybir.ActivationFunctionType.Copy
    mult_op = mybir.AluOpType.mult
    from concourse.tile_rust import add_dep_helper
    first_evac = [None]

    with tile.TileContext(nc) as tc, ExitStack() as ctx:
        # ---- pools ----
        resident = ctx.enter_context(tc.tile_pool(name="resident", bufs=1))
        biaspool = ctx.enter_context(tc.tile_pool(name="bias", bufs=4))
        exppool = ctx.enter_context(tc.tile_pool(name="exps", bufs=5))
        outpool = ctx.enter_context(tc.tile_pool(name="outs", bufs=3))
        smallpool = ctx.enter_context(tc.tile_pool(name="small", bufs=4))
        ps_main = ctx.enter_context(
            tc.tile_pool(name="ps_main", bufs=2, space="PSUM"))
        ps_pv = ctx.enter_context(
            tc.tile_pool(name="ps_pv", bufs=2, space="PSUM"))

        # ---- resident tiles ----
        # chunked + consumption-ordered loads: DMA queues are global FIFOs,
        # so enqueue bytes in the order the projections will need them
        xts = resident.tile([128, CT, N], f16)          # xT tiled on c
        wq_sb = resident.tile([128, CT, JL], f16)       # wqT tiled on c
        wk_sb = resident.tile([128, CT, JL], f16)       # wkT tiled on c
        wv_sb = resident.tile([128, CT, JL], f16)       # wvT tiled on c
        wo_sb = resident.tile([128, 4, C], f16)         # woT tiled on j
        xview = xT.rearrange("(ct p) n -> p ct n", p=128)
        wqview = wqT.rearrange("(ct p) j -> p ct j", p=128)
        wkview = wkT.rearrange("(ct p) j -> p ct j", p=128)
        d0 = None
        for ct in range(CT):
            nc.sync.dma_start(wq_sb[:, ct, 0:256], wqview[:, ct, 0:256])
            last = nc.sync.dma_start(xts[:, ct, :], xview[:, ct, :])
            if d0 is None:
                d0 = last
            # wk jt0-half rides along per-ct so k-jp0 never waits on bulk
            nc.sync.dma_start(wk_sb[:, ct, 0:256], wkview[:, ct, 0:256])
        # consumption order: q-jp1, k-jp1, v
        bulk = [
            nc.sync.dma_start(wq_sb[:, :, 256:512], wqview[:, :, 256:512]),
            nc.sync.dma_start(wk_sb[:, :, 256:512], wkview[:, :, 256:512]),
            nc.sync.dma_start(wv_sb[:],
                              wvT.rearrange("(ct p) j -> p ct j", p=128)),
            nc.sync.dma_start(wo_sb[:],
                              woT.rearrange("(kt p) c -> p kt c", p=128)),
        ]
        for b in bulk:
            add_dep_helper(b.ins, d0.ins, reason="startup DMA priority")

        id_sb = resident.tile([128, 128], f16)
        nc.sync.dma_start(id_sb[:], ident)

        madd_sb = resident.tile([128, NT], f32)
        nc.sync.dma_start(madd_sb[:], madd)

        bq_sb = resident.tile([1, JL], f16)
        nc.sync.dma_start(bq_sb[:], bqr)
        bk_sb = resident.tile([1, JL], f16)
        nc.sync.dma_start(bk_sb[:], bkr)

        qT_sb = resident.tile([128, 4, N], f16)         # [j-tile, n]
        # k stored zero-padded per head: rows 0:64 = kT_h, rows 64:128 = 0,
        # so the S stationary is always a full (128,128) tile -- the PE
        # charges ~105ns to reconfigure whenever the stationary tile size
        # changes, and this keeps every matmul in (128,128) mode.
        kZ_sb = resident.tile([128, HL, N], f16)
        # v with 64 ones-columns per head: [m-tile, h*(64 V + 64 ones)]
        v_sb = resident.tile([128, NT, HL * 2 * D], f16)
        # attT as one tile per head-pair (o_proj stationary layout)
        attT_t = [resident.tile([128, N], f16, name=f"attT_{i}")
                  for i in range(4)]

        ones_f32 = resident.tile([128, 1], f32)
        nc.vector.memset(ones_f32[:], 1.0)
        ones_row = None
        if with_qk_bias:
            ones_row = resident.tile([1, N], f16)
            nc.vector.tensor_copy(
                ones_row[:], ones_f32[0:1, 0:1].to_broadcast([1, N]))

        # even heads: kT in rows 0:64 (q rows 0:64), zeros below;
        # odd heads: kT in rows 64:128 (q rows 64:128), zeros above
        nc.gpsimd.memset(kZ_sb[64:128, 0:HL:2, :], 0.0)
        nc.gpsimd.memset(kZ_sb[0:64, 1:HL:2, :], 0.0)

        # ones columns of v (wide softmax-denominator trick)
        for mt in range(NT):
            v4o = v_sb[:, mt, :].rearrange("p (h c) -> p h c", c=2 * D)
            nc.vector.tensor_copy(
                v4o[:, :, D:2 * D],
                ones_f32[:, 0:1, None].to_broadcast([128, HL, D]))



        # ---- PE warm-up while the first DMAs land ----
        warm_sb = resident.tile([128, 512], f16)
        nc.vector.memset(warm_sb[:], 0.0)
        warm_ps = ps_pv.tile([128, 512], f32, tag="pv", name="warm")
        for i in range(4):
            nc.tensor.matmul(warm_ps[:], warm_sb[:, 0:128], warm_sb[:],
                             start=True, stop=True)

        # ---- phase A: projections, 4-deep across both PSUM pools ----
        evac_ctr = [0]

        def evac(dst, src):
            # alternate DVE / ACT so neither engine gates the projections
            ev = evac_ctr[0]
            evac_ctr[0] += 1
            if ev % 2 == 0:
                return nc.vector.tensor_copy(dst, src)
            return nc.scalar.activation(dst, src, Copy)

        def emit_qk_group(wsb, brow, dest, jp):
            ps2 = [ps_main.tile([128, N], f32, tag="mm", name=f"proj_{jp}_0"),
                   ps_pv.tile([128, N], f32, tag="pv", name=f"proj_{jp}_1")]
            for ct in range(CT):
                for ji in range(2):
                    jt = jp * 2 + ji
                    w = wsb[:, ct, jt * 128:(jt + 1) * 128]
                    for nh in range(2):
                        nc.tensor.matmul(
                            ps2[ji][:, nh * 512:(nh + 1) * 512],
                            w[:],
                            xts[:, ct, nh * 512:(nh + 1) * 512],
                            start=(ct == 0),
                            stop=(not with_qk_bias and ct == CT - 1))
            for ji in range(2):
                jt = jp * 2 + ji
                if with_qk_bias:
                    for nh in range(2):
                        nc.tensor.matmul(
                            ps2[ji][:, nh * 512:(nh + 1) * 512],
                            brow[0:1, jt * 128:(jt + 1) * 128],
                            ones_row[0:1, nh * 512:(nh + 1) * 512],
                            start=False, stop=True)
                ev = evac(dest[:, jt, :], ps2[ji][:])
                if first_evac[0] is None:
                    first_evac[0] = ev

        def emit_v_group(mt, pool, tag):
            # v normal layout: out[m-tile, j] = sum_c xT[c, m] * wvT[c, j]
            ps = pool.tile([128, N], f32, tag=tag, name=f"vproj_{mt}")
            psv = ps[:, 0:JL]
            for ct in range(CT):
                nc.tensor.matmul(
                    psv,
                    xts[:, ct, mt * 128:(mt + 1) * 128],
                    wv_sb[:, ct, :],
                    start=(ct == 0), stop=(ct == CT - 1))
            v4 = v_sb[:, mt, :].rearrange("p (h c) -> p h c", c=2 * D)
            evac(v4[:, :, 0:D], psv.rearrange("p (h c) -> p h c", c=D))

        def emit_k_group(jp):
            ps2 = [ps_main.tile([128, N], f32, tag="mm", name=f"kproj_{jp}_0"),
                   ps_pv.tile([128, N], f32, tag="pv", name=f"kproj_{jp}_1")]
            for ct in range(CT):
                for ji in range(2):
                    jt = jp * 2 + ji
                    w = wk_sb[:, ct, jt * 128:(jt + 1) * 128]
                    for nh in range(2):
                        nc.tensor.matmul(
                            ps2[ji][:, nh * 512:(nh + 1) * 512],
                            w[:],
                            xts[:, ct, nh * 512:(nh + 1) * 512],
                            start=(ct == 0),
                            stop=(not with_qk_bias and ct == CT - 1))
            for ji in range(2):
                jt = jp * 2 + ji
                if with_qk_bias:
                    for nh in range(2):
                        nc.tensor.matmul(
                            ps2[ji][:, nh * 512:(nh + 1) * 512],
                            bk_sb[0:1, jt * 128:(jt + 1) * 128],
                            ones_row[0:1, nh * 512:(nh + 1) * 512],
                            start=False, stop=True)
                evac(kZ_sb[0:64, 2 * jt, :], ps2[ji][0:64, :])
                evac(kZ_sb[64:128, 2 * jt + 1, :], ps2[ji][64:128, :])

        emit_qk_group(wq_sb, bq_sb, qT_sb, 0)
        emit_k_group(0)
        emit_qk_group(wq_sb, bq_sb, qT_sb, 1)
        emit_k_group(1)
        for mt in range(NT):
            pool, tag = ((ps_pv, "pv") if mt % 2 else (ps_main, "mm"))
            emit_v_group(mt, pool, tag)

        # ---- phase B: attention, one head at a time ----
        # 3-deep S-psum rotation + lag-2 PV: every cross-engine edge the
        # in-order PE stream waits on is >= 2 steps old, so the PE never
        # idles (which would demote the p-state).
        bias_view = biasT.rearrange("h (t p) n -> h p t n", p=128)

        def emit_scores(h, mt, eb, par):
            sp = ps_main.tile([128, N], f32, tag="mm", name=f"s_{h}_{mt}")
            jt = h // 2
            # keep-warm filler: with all matmuls in (128,128) mode the PE
            # per-step cost (1065ns) would dip under ACT's exp (1095ns) and
            # idle slivers would demote the p-state; a 256-col dummy into
            # the about-to-be-reset psum keeps the PE strictly bottleneck
            # hybrid bias add: n-half 0 lands in PSUM via identity matmul;
            # n-half 1 is folded as exp(S)*exp(B) by the DVE multiply below
            nc.tensor.matmul(sp[:, 0:512], id_sb[:], eb[:, par, 0:512],
                             start=True, stop=False, skip_group_check=True)
            nc.tensor.matmul(
                sp[:, 0:512],
                kZ_sb[:, h, mt * 128:(mt + 1) * 128],
                qT_sb[:, jt, 0:512],
                start=False, stop=True, skip_group_check=True)
            nc.tensor.matmul(
                sp[:, 512:1024],
                kZ_sb[:, h, mt * 128:(mt + 1) * 128],
                qT_sb[:, jt, 512:1024],
                start=True, stop=True)
            et = exppool.tile([128, N], f16, tag="exp", name=f"exp_{h}_{mt}")
            nc.scalar.activation(et[:], sp[:], Exp,
                                 bias=madd_sb[:, mt:mt + 1])
            nc.vector.tensor_tensor(et[:, 512:1024], et[:, 512:1024],
                                    eb[:, par, 512:1024], mult_op)
            return et

        def emit_pv(h, mt, pv, et):
            vx = v_sb[:, mt, h * 128:(h + 1) * 128]
            for nh in range(2):
                sl = slice(nh * 512, (nh + 1) * 512)
                nc.tensor.matmul(
                    pv[:, sl], vx, et[:, sl],
                    start=(mt == 0), stop=(mt == NT - 1))

        def emit_normalize(h, pv):
            # rows 64:127 of pv hold the denominator replicated; DVE-only
            # chain so the PE stream has nothing to stall on here.  For the
            # last head this chain is the kernel's tail critical path: the
            # body copy moves to the (by then idle) ACT engine and the
            # final multiply is split so o_proj's first kt=3 matmuls can
            # start after the first half.
            last = (h == HL - 1)
            dst = attT_t[h // 2][(h % 2) * 64:(h % 2) * 64 + 64, :]
            if last:
                # per-half chain so o_proj's kt=3 matmuls for the first
                # n-tiles can start after ~half the normalize latency
                for nh in range(2):
                    sl = slice(nh * 512, (nh + 1) * 512)
                    d_ = smallpool.tile([64, 512], f32, tag="den",
                                        name=f"den_{h}_{nh}")
                    nc.vector.tensor_copy(d_[:], pv[64:128, sl])
                    r = smallpool.tile([64, 512], f32, tag="recip",
                                       name=f"recip_{h}_{nh}")
                    nc.vector.reciprocal_approx_fast(out=r[:], in_=d_[:])
                    nc.vector.tensor_tensor(dst[:, sl], pv[0:64, sl],
                                            r[:], mult_op)
            else:
                d_ = smallpool.tile([64, N], f32, tag="den", name=f"den_{h}")
                nc.vector.tensor_copy(d_[:], pv[64:128, :])
                r = smallpool.tile([64, N], f32, tag="recip",
                                   name=f"recip_{h}")
                nc.vector.reciprocal_approx_fast(out=r[:], in_=d_[:])
                # multiply straight off the PV PSUM (no body copy)
                nc.vector.tensor_tensor(dst, pv[0:64, :], r[:], mult_op)

        pv_by_head = {}
        eb_cur = [None]
        pending = []
        for g in range(HL * NT):
            h, mt = divmod(g, NT)
            if mt == 0:
                pv_by_head[h] = ps_pv.tile([128, N], f32, tag="pv",
                                           name=f"pv_{h}")
            if mt % 4 == 0:
                eb = biaspool.tile([128, 4, N], f16, tag="bias",
                                   name=f"bias_{h}_{mt}")
                bdma = nc.gpsimd.dma_start(
                    eb[:], bias_view[h, :, mt:mt + 4, :])
                if h == 0 and first_evac[0] is not None:
                    add_dep_helper(bdma.ins, first_evac[0].ins,
                                   reason="bias prefetch behind startup")
                eb_cur[0] = eb
            et = emit_scores(h, mt, eb_cur[0], mt % 4)
            pending.append((h, mt, et))
            if g >= 3:
                ph, pmt, pet = pending.pop(0)
                emit_pv(ph, pmt, pv_by_head[ph], pet)
                if pmt == NT - 1:
                    emit_normalize(ph, pv_by_head[ph])
        while pending:
            ph, pmt, pet = pending.pop(0)
            emit_pv(ph, pmt, pv_by_head[ph], pet)
            if pmt == NT - 1:
                emit_normalize(ph, pv_by_head[ph])

        # ---- phase C: o_proj partial ----
        # kt-outer so both ch matmuls share the same stationary attT slice.
        # Three groups open with kt=0..2 before any kt=3, so the PE stream
        # has ~4us of ready work while the last pair's normalize (DVE)
        # finishes -- no idle, no p-state demotion.
        def open_group(nt):
            pool, tag = ((ps_pv, "pv") if nt % 2 else (ps_main, "mm"))
            ps = pool.tile([128, N], f32, tag=tag, name=f"oproj_{nt}")
            for kt in range(3):
                att = attT_t[kt][:, nt * 128:(nt + 1) * 128]
                for ch in range(2):
                    nc.tensor.matmul(
                        ps[:, ch * 512:(ch + 1) * 512],
                        att,
                        wo_sb[:, kt, ch * 512:(ch + 1) * 512],
                        start=(kt == 0), stop=False)
            return ps

        def close_group(nt, ps):
            att = attT_t[3][:, nt * 128:(nt + 1) * 128]
            for ch in range(2):
                nc.tensor.matmul(
                    ps[:, ch * 512:(ch + 1) * 512],
                    att,
                    wo_sb[:, 3, ch * 512:(ch + 1) * 512],
                    start=False, stop=True)
            ot = outpool.tile([128, N], f16, tag="out")
            if nt == NT - 1:
                # split the last tile's evac+store so the DMA of the first
                # half overlaps the evacuation of the second (shorter tail)
                for h2 in range(2):
                    sl = slice(h2 * 512, (h2 + 1) * 512)
                    (nc.scalar.activation(ot[:, sl], ps[:, sl], Copy)
                     if h2 == 0 else
                     nc.vector.tensor_copy(ot[:, sl], ps[:, sl]))
                    nc.sync.dma_start(outp[nt * 128:(nt + 1) * 128, sl],
                                      ot[:, sl])
            else:
                if nt % 2 == 0:
                    nc.scalar.activation(ot[:], ps[:], Copy)
                else:
                    nc.vector.tensor_copy(ot[:], ps[:])
                nc.sync.dma_start(outp[nt * 128:(nt + 1) * 128, :], ot[:])

        open_ps = [open_group(0), open_group(1)]
        for nt in range(NT):
            close_group(nt, open_ps[nt])
            if nt + 2 < NT:
                open_ps.append(open_group(nt + 2))

    nc.compile()
    return nc


def get_program(with_qk_bias=True):
    key = ("nc", with_qk_bias)
    if key not in _prog_cache:
        _prog_cache[key] = build_program(with_qk_bias=with_qk_bias)
    return _prog_cache[key]


def make_in_maps(x, attn_bias, attn_mask, Wq, bq, Wk, bk, Wv, bv, Wo, bo):
    """Host-side shard + layout prep.  Returns (in_maps, const) where
    const[c_out] = bo + bv @ Wo.T must be added to the gathered output."""
    x = np.asarray(x, np.float32)
    attn_bias = np.asarray(attn_bias, np.float32)
    attn_mask = np.asarray(attn_mask)
    Wq = np.asarray(Wq, np.float32)
    Wk = np.asarray(Wk, np.float32)
    Wv = np.asarray(Wv, np.float32)
    Wo = np.asarray(Wo, np.float32)
    bq = np.asarray(bq, np.float32)
    bk = np.asarray(bk, np.float32)
    bv = np.asarray(bv, np.float32)
    bo = np.asarray(bo, np.float32)

    const = bo + bv @ Wo.T
    ident = np.eye(128, dtype=np.float16)

    xTs = [np.ascontiguousarray(x[b].T).astype(np.float16) for b in range(B)]
    madds = []
    for b in range(B):
        ma = np.where(attn_mask[b] == 0, np.float32(-1e30), np.float32(0.0))
        madds.append(np.ascontiguousarray(ma.reshape(NT, 128).T))

    in_maps = []
    for core in range(8):
        b, half = divmod(core, 2)
        rows = slice(half * JL, (half + 1) * JL)
        wqT = np.ascontiguousarray(
            (Wq[rows, :] * np.float32(SCALE)).T).astype(np.float16)
        wkT = np.ascontiguousarray(Wk[rows, :].T).astype(np.float16)
        wvT = np.ascontiguousarray(Wv[rows, :].T).astype(np.float16)
        woT = np.ascontiguousarray(Wo[:, rows].T).astype(np.float16)
        bqr = (bq[rows] * np.float32(SCALE)).reshape(1, JL).astype(np.float16)
        bkr = bk[rows].reshape(1, JL).astype(np.float16)
        bT32 = np.ascontiguousarray(
            attn_bias[b, half * HL:(half + 1) * HL].transpose(0, 2, 1))
        # n-half 0 raw (added via identity matmul), n-half 1 exponentiated
        # (folded as exp(S)*exp(B) on DVE)
        bT32[:, :, N // 2:] = np.exp(bT32[:, :, N // 2:])
        bT = bT32.astype(np.float16)
        in_maps.append({
            "xT": xTs[b], "wqT": wqT, "wkT": wkT, "wvT": wvT, "woT": woT,
            "bqr": bqr, "bkr": bkr, "ident": ident, "biasT": bT,
            "madd": madds[b],
        })
    return in_maps, const


def gather(results, const):
    out = np.empty((B, N, C), np.float32)
    for b in range(B):
        out[b] = results[2 * b]["outp"].astype(np.float32) \
            + results[2 * b + 1]["outp"].astype(np.float32) \
            + const[None, :]
    return out


def kernel(**inputs):
    from concourse.bass_utils import run_bass_kernel_spmd
    wb = bool(np.any(np.asarray(inputs["bq"]))
              or np.any(np.asarray(inputs["bk"])))
    nc = get_program(with_qk_bias=wb)
    in_maps, const = make_in_maps(**inputs)
    res = run_bass_kernel_spmd(nc, in_maps, core_ids=list(range(8)))
    return gather(res.results, const)


# revision 31
# speedup vs baseline: 1.0315x; 1.0061x over previous
"""Trainium2 Bass kernel for BiasedMHA (B=4, N=1024, C=1024, H=16, D=64).

Sharding: 8 cores = 4 batches x 2 head-halves (8 heads each).
Each core computes q/k/v projections for its head slice, biased softmax
attention, and its partial o_proj.  Host sums the two partials per batch
and adds the (bo + bv @ Wo.T) constant.

v8: PE-density-first + latency-loop-free pipeline.  The PE p-state ramp
demotes to ~2x cycle time after ANY idle and needs >3us of continuous
execution to recover, so the whole kernel keeps the PE the strict
bottleneck with deep enough buffering that no WAR/RAW edge can stall
the in-order PE stream:
- attention processes heads SINGLY (not in pairs): the PV accumulator
  then needs only one [128,1024] PSUM tile (2 banks), freeing budget
  for THREE S-psum tiles (6 banks).  With a 3-deep S rotation and
  lag-2 PV emission every cross-engine dependency is >= 2 steps old.
- attn_bias lands in PSUM via identity matmul (start=True): useful PE
  filler; exp reads S+B straight from PSUM.
- wide denominator: the PV stationary carries 64 ones-columns, so PSUM
  rows 64:127 hold the softmax denominator replicated.  Normalize is
  den-copy/body-copy/recip/mult on DVE only.
- projections pipeline 4-deep across both PSUM pools; evacuations
  alternate DVE / ACT-Copy; DMA arrival order matches consumption.

Layouts (host-prepped, contraction-on-partitions):
  xT    [C, N]  fp16 : x[b].T
  wqT   [C, 512] fp16: Wq[rows,:].T * scale (softmax scale folded)
  wkT/wvT [C, 512] fp16
  woT   [512, C] fp16: Wo[:, cols].T
  biasT [8, N, N] fp16: attn_bias[b, local head][query, key].T
  bqr/bkr [1, 512] fp16: bias rows via K=1 matmuls (skipped when the
        host detects all-zero q/k biases)
  ident [128, 128] fp16: identity (stationary of the bias-add matmul)
  madd  [128, NT] fp32: additive mask (-1e30 where attn_mask==0)
"""

import sys

if "/opt/trn_rl_repo" not in sys.path:
    sys.path.insert(0, "/opt/trn_rl_repo")

from contextlib import ExitStack

import numpy as np

B, N, C, H = 4, 1024, 1024, 16
D = C // H            # 64
HL = H // 2           # 8 local heads per core
JL = HL * D           # 512 local head dims
NT = N // 128         # 8 seq tiles
CT = C // 128         # 8 contraction tiles
SCALE = D ** (-0.5)

_prog_cache = {}


def build_program(with_qk_bias=True):
    import concourse.tile as tile
    from concourse import bacc, mybir
    f32 = mybir.dt.float32
    f16 = mybir.dt.float16

    nc = bacc.Bacc("TRN2", target_bir_lowering=False, debug=False,
                   enable_asserts=False, num_devices=8)

    xT = nc.dram_tensor("xT", [C, N], f16, kind="ExternalInput").ap()
    wqT = nc.dram_tensor("wqT", [C, JL], f16, kind="ExternalInput").ap()
    wkT = nc.dram_tensor("wkT", [C, JL], f16, kind="ExternalInput").ap()
    wvT = nc.dram_tensor("wvT", [C, JL], f16, kind="ExternalInput").ap()
    woT = nc.dram_tensor("woT", [JL, C], f16, kind="ExternalInput").ap()
    bqr = nc.dram_tensor("bqr", [1, JL], f16, kind="ExternalInput").ap()
    bkr = nc.dram_tensor("bkr", [1, JL], f16, kind="ExternalInput").ap()
    ident = nc.dram_tensor("ident", [128, 128], f16, kind="ExternalInput").ap()
    biasT = nc.dram_tensor("biasT", [HL, N, N], f16,
                           kind="ExternalInput").ap()
    madd = nc.dram_tensor("madd", [128, NT], f32, kind="ExternalInput").ap()
    outp = nc.dram_tensor("outp", [N, C], f16, kind="ExternalOutput").ap()

    Exp = mybir.ActivationFunctionType.Exp
    Copy = mybir.ActivationFunctionType.Copy
    mult_op = mybir.AluOpType.mult
    from concourse.tile_rust import add_dep_helper
    first_evac = [None]

    with tile.TileContext(nc) as tc, ExitStack() as ctx:
        # ---- pools ----
        resident = ctx.enter_context(tc.tile_pool(name="resident", bufs=1))
        biaspool = ctx.enter_context(tc.tile_pool(name="bias", bufs=4))
        exppool = ctx.enter_context(tc.tile_pool(name="exps", bufs=6))
        outpool = ctx.enter_context(tc.tile_pool(name="outs", bufs=3))
        smallpool = ctx.enter_context(tc.tile_pool(name="small", bufs=4))
        ps_main = ctx.enter_context(
            tc.tile_pool(name="ps_main", bufs=2, space="PSUM"))
        ps_pv = ctx.enter_context(
            tc.tile_pool(name="ps_pv", bufs=2, space="PSUM"))

        # ---- resident tiles ----
        # chunked + consumption-ordered loads: DMA queues are global FIFOs,
        # so enqueue bytes in the order the projections will need them
        xts = resident.tile([128, CT, N], f16)          # xT tiled on c
        wq_sb = resident.tile([128, CT, JL], f16)       # wqT tiled on c
        wk_sb = resident.tile([128, CT, JL], f16)       # wkT tiled on c
        wv_sb = resident.tile([128, CT, JL], f16)       # wvT tiled on c
        wo_sb = resident.tile([128, 4, C], f16)         # woT tiled on j
        xview = xT.rearrange("(ct p) n -> p ct n", p=128)
        wqview = wqT.rearrange("(ct p) j -> p ct j", p=128)
        wkview = wkT.rearrange("(ct p) j -> p ct j", p=128)
        d0 = None
        for ct in range(CT):
            nc.sync.dma_start(wq_sb[:, ct, 0:256], wqview[:, ct, 0:256])
            last = nc.sync.dma_start(xts[:, ct, :], xview[:, ct, :])
            if d0 is None:
                d0 = last
            # wk jt0-half rides along per-ct so k-jp0 never waits on bulk
            nc.sync.dma_start(wk_sb[:, ct, 0:256], wkview[:, ct, 0:256])
        # consumption order: q-jp1, k-jp1, v
        bulk = [
            nc.sync.dma_start(wq_sb[:, :, 256:512], wqview[:, :, 256:512]),
            nc.sync.dma_start(wk_sb[:, :, 256:512], wkview[:, :, 256:512]),
            nc.sync.dma_start(wv_sb[:],
                              wvT.rearrange("(ct p) j -> p ct j", p=128)),
            nc.sync.dma_start(wo_sb[:],
                              woT.rearrange("(kt p) c -> p kt c", p=128)),
        ]
        for b in bulk:
            add_dep_helper(b.ins, d0.ins, reason="startup DMA priority")

        id_sb = resident.tile([128, 128], f16)
        nc.sync.dma_start(id_sb[:], ident)

        madd_sb = resident.tile([128, NT], f32)
        nc.sync.dma_start(madd_sb[:], madd)

        bq_sb = resident.tile([1, JL], f16)
        nc.sync.dma_start(bq_sb[:], bqr)
        bk_sb = resident.tile([1, JL], f16)
        nc.sync.dma_start(bk_sb[:], bkr)

        qT_sb = resident.tile([128, 4, N], f16)         # [j-tile, n]
        # k stored zero-padded per head: rows 0:64 = kT_h, rows 64:128 = 0,
        # so the S stationary is always a full (128,128) tile -- the PE
        # charges ~105ns to reconfigure whenever the stationary tile size
        # changes, and this keeps every matmul in (128,128) mode.
        kZ_sb = resident.tile([128, HL, N], f16)
        # v with 64 ones-columns per head: [m-tile, h*(64 V + 64 ones)]
        v_sb = resident.tile([128, NT, HL * 2 * D], f16)
        # attT as one tile per head-pair (o_proj stationary layout)
        attT_t = [resident.tile([128, N], f16, name=f"attT_{i}")
                  for i in range(4)]

        ones_f32 = resident.tile([128, 1], f32)
        nc.vector.memset(ones_f32[:], 1.0)
        ones_row = None
        if with_qk_bias:
            ones_row = resident.tile([1, N], f16)
            nc.vector.tensor_copy(
                ones_row[:], ones_f32[0:1, 0:1].to_broadcast([1, N]))

        # even heads: kT in rows 0:64 (q rows 0:64), zeros below;
        # odd heads: kT in rows 64:128 (q rows 64:128), zeros above
        nc.gpsimd.memset(kZ_sb[64:128, 0:HL:2, :], 0.0)
        nc.gpsimd.memset(kZ_sb[0:64, 1:HL:2, :], 0.0)

        # ones columns of v (wide softmax-denominator trick)
        for mt in range(NT):
            v4o = v_sb[:, mt, :].rearrange("p (h c) -> p h c", c=2 * D)
            nc.vector.tensor_copy(
                v4o[:, :, D:2 * D],
                ones_f32[:, 0:1, None].to_broadcast([128, HL, D]))



        # ---- PE warm-up while the first DMAs land ----
        warm_sb = resident.tile([128, 512], f16)
        nc.vector.memset(warm_sb[:], 0.0)
        warm_ps = ps_pv.tile([128, 512], f32, tag="pv", name="warm")
        for i in range(4):
            nc.tensor.matmul(warm_ps[:], warm_sb[:, 0:128], warm_sb[:],
                             start=True, stop=True)

        # ---- phase A: projections, 4-deep across both PSUM pools ----
        evac_ctr = [0]

        def evac(dst, src):
            # alternate DVE / ACT so neither engine gates the projections
            ev = evac_ctr[0]
            evac_ctr[0] += 1
            if ev % 2 == 0:
                return nc.vector.tensor_copy(dst, src)
            return nc.scalar.activation(dst, src, Copy)

        def emit_qk_group(wsb, brow, dest, jp):
            ps2 = [ps_main.tile([128, N], f32, tag="mm", name=f"proj_{jp}_0"),
                   ps_pv.tile([128, N], f32, tag="pv", name=f"proj_{jp}_1")]
            for ct in range(CT):
                for ji in range(2):
                    jt = jp * 2 + ji
                    w = wsb[:, ct, jt * 128:(jt + 1) * 128]
                    for nh in range(2):
                        nc.tensor.matmul(
                            ps2[ji][:, nh * 512:(nh + 1) * 512],
                            w[:],
                            xts[:, ct, nh * 512:(nh + 1) * 512],
                            start=(ct == 0),
                            stop=(not with_qk_bias and ct == CT - 1))
            for ji in range(2):
                jt = jp * 2 + ji
                if with_qk_bias:
                    for nh in range(2):
                        nc.tensor.matmul(
                            ps2[ji][:, nh * 512:(nh + 1) * 512],
                            brow[0:1, jt * 128:(jt + 1) * 128],
                            ones_row[0:1, nh * 512:(nh + 1) * 512],
                            start=False, stop=True)
                ev = evac(dest[:, jt, :], ps2[ji][:])
                if first_evac[0] is None:
                    first_evac[0] = ev

        def emit_v_group(mt, pool, tag):
            # v normal layout: out[m-tile, j] = sum_c xT[c, m] * wvT[c, j]
            ps = pool.tile([128, N], f32, tag=tag, name=f"vproj_{mt}")
            psv = ps[:, 0:JL]
            for ct in range(CT):
                nc.tensor.matmul(
                    psv,
                    xts[:, ct, mt * 128:(mt + 1) * 128],
                    wv_sb[:, ct, :],
                    start=(ct == 0), stop=(ct == CT - 1))
            v4 = v_sb[:, mt, :].rearrange("p (h c) -> p h c", c=2 * D)
            evac(v4[:, :, 0:D], psv.rearrange("p (h c) -> p h c", c=D))

        def emit_k_group(jp):
            ps2 = [ps_main.tile([128, N], f32, tag="mm", name=f"kproj_{jp}_0"),
                   ps_pv.tile([128, N], f32, tag="pv", name=f"kproj_{jp}_1")]
            for ct in range(CT):
                for ji in range(2):
                    jt = jp * 2 + ji
                    w = wk_sb[:, ct, jt * 128:(jt + 1) * 128]
                    for nh in range(2):
                        nc.tensor.matmul(
                            ps2[ji][:, nh * 512:(nh + 1) * 512],
                            w[:],
                            xts[:, ct, nh * 512:(nh + 1) * 512],
                            start=(ct == 0),
                            stop=(not with_qk_bias and ct == CT - 1))
            for ji in range(2):
                jt = jp * 2 + ji
                if with_qk_bias:
                    for nh in range(2):
                        nc.tensor.matmul(
                            ps2[ji][:, nh * 512:(nh + 1) * 512],
                            bk_sb[0:1, jt * 128:(jt + 1) * 128],
                            ones_row[0:1, nh * 512:(nh + 1) * 512],
                            start=False, stop=True)
                evac(kZ_sb[0:64, 2 * jt, :], ps2[ji][0:64, :])
                evac(kZ_sb[64:128, 2 * jt + 1, :], ps2[ji][64:128, :])

        emit_qk_group(wq_sb, bq_sb, qT_sb, 0)
        emit_k_group(0)
        emit_qk_group(wq_sb, bq_sb, qT_sb, 1)
        emit_k_group(1)
        for mt in range(NT):
            pool, tag = ((ps_pv, "pv") if mt % 2 else (ps_main, "mm"))
            emit_v_group(mt, pool, tag)

        # ---- phase B: attention, one head at a time ----
        # 3-deep S-psum rotation + lag-2 PV: every cross-engine edge the
        # in-order PE stream waits on is >= 2 steps old, so the PE never
        # idles (which would demote the p-state).
        bias_view = biasT.rearrange("h (t p) n -> h p t n", p=128)

        def emit_scores(h, mt, eb, par):
            sp = ps_main.tile([128, N], f32, tag="mm", name=f"s_{h}_{mt}")
            jt = h // 2
            # keep-warm filler: with all matmuls in (128,128) mode the PE
            # per-step cost (1065ns) would dip under ACT's exp (1095ns) and
            # idle slivers would demote the p-state; a 256-col dummy into
            # the about-to-be-reset psum keeps the PE strictly bottleneck
            # hybrid bias add: n-half 0 lands in PSUM via identity matmul;
            # n-half 1 is folded as exp(S)*exp(B) by the DVE multiply below
            nc.tensor.matmul(sp[:, 0:512], id_sb[:], eb[:, par, 0:512],
                             start=True, stop=False, skip_group_check=True)
            nc.tensor.matmul(
                sp[:, 0:512],
                kZ_sb[:, h, mt * 128:(mt + 1) * 128],
                qT_sb[:, jt, 0:512],
                start=False, stop=True, skip_group_check=True)
            nc.tensor.matmul(
                sp[:, 512:1024],
                kZ_sb[:, h, mt * 128:(mt + 1) * 128],
                qT_sb[:, jt, 512:1024],
                start=True, stop=True)
            et = exppool.tile([128, N], f16, tag="exp", name=f"exp_{h}_{mt}")
            nc.scalar.activation(et[:], sp[:], Exp,
                                 bias=madd_sb[:, mt:mt + 1])
            nc.vector.tensor_tensor(et[:, 512:1024], et[:, 512:1024],
                                    eb[:, par, 512:1024], mult_op)
            return et

        def emit_pv(h, mt, pv, et):
            vx = v_sb[:, mt, h * 128:(h + 1) * 128]
            for nh in range(2):
                sl = slice(nh * 512, (nh + 1) * 512)
                nc.tensor.matmul(
                    pv[:, sl], vx, et[:, sl],
                    start=(mt == 0), stop=(mt == NT - 1))

        def emit_normalize(h, pv):
            # rows 64:127 of pv hold the denominator replicated; DVE-only
            # chain so the PE stream has nothing to stall on here.  For the
            # last head this chain is the kernel's tail critical path: the
            # body copy moves to the (by then idle) ACT engine and the
            # final multiply is split so o_proj's first kt=3 matmuls can
            # start after the first half.
            last = (h == HL - 1)
            dst = attT_t[h // 2][(h % 2) * 64:(h % 2) * 64 + 64, :]
            if last:
                # per-half chain so o_proj's kt=3 matmuls for the first
                # n-tiles can start after ~half the normalize latency
                for nh in range(2):
                    sl = slice(nh * 512, (nh + 1) * 512)
                    d_ = smallpool.tile([64, 512], f32, tag="den",
                                        name=f"den_{h}_{nh}")
                    nc.vector.tensor_copy(d_[:], pv[64:128, sl])
                    r = smallpool.tile([64, 512], f32, tag="recip",
                                       name=f"recip_{h}_{nh}")
                    nc.vector.reciprocal_approx_fast(out=r[:], in_=d_[:])
                    nc.vector.tensor_tensor(dst[:, sl], pv[0:64, sl],
                                            r[:], mult_op)
            else:
                d_ = smallpool.tile([64, N], f32, tag="den", name=f"den_{h}")
                nc.vector.tensor_copy(d_[:], pv[64:128, :])
                r = smallpool.tile([64, N], f32, tag="recip",
                                   name=f"recip_{h}")
                nc.vector.reciprocal_approx_fast(out=r[:], in_=d_[:])
                # multiply straight off the PV PSUM (no body copy)
                nc.vector.tensor_tensor(dst, pv[0:64, :], r[:], mult_op)

        pv_by_head = {}
        eb_cur = [None]
        pending = []
        for g in range(HL * NT):
            h, mt = divmod(g, NT)
            if mt == 0:
                pv_by_head[h] = ps_pv.tile([128, N], f32, tag="pv",
                                           name=f"pv_{h}")
            if mt % 4 == 0:
                eb = biaspool.tile([128, 4, N], f16, tag="bias",
                                   name=f"bias_{h}_{mt}")
                bdma = nc.gpsimd.dma_start(
                    eb[:], bias_view[h, :, mt:mt + 4, :])
                if h == 0 and first_evac[0] is not None:
                    add_dep_helper(bdma.ins, first_evac[0].ins,
                                   reason="bias prefetch behind startup")
                eb_cur[0] = eb
            et = emit_scores(h, mt, eb_cur[0], mt % 4)
            pending.append((h, mt, et))
            if g >= 3:
                ph, pmt, pet = pending.pop(0)
                emit_pv(ph, pmt, pv_by_head[ph], pet)
                if pmt == NT - 1:
                    emit_normalize(ph, pv_by_head[ph])
        while pending:
            ph, pmt, pet = pending.pop(0)
            emit_pv(ph, pmt, pv_by_head[ph], pet)
            if pmt == NT - 1:
                emit_normalize(ph, pv_by_head[ph])

        # ---- phase C: o_proj partial ----
        # kt-outer so both ch matmuls share the same stationary attT slice.
        # Three groups open with kt=0..2 before any kt=3, so the PE stream
        # has ~4us of ready work while the last pair's normalize (DVE)
        # finishes -- no idle, no p-state demotion.
        def open_group(nt):
            pool, tag = ((ps_pv, "pv") if nt % 2 else (ps_main, "mm"))
            ps = pool.tile([128, N], f32, tag=tag, name=f"oproj_{nt}")
            for kt in range(3):
                att = attT_t[kt][:, nt * 128:(nt + 1) * 128]
                for ch in range(2):
                    nc.tensor.matmul(
                        ps[:, ch * 512:(ch + 1) * 512],
                        att,
                        wo_sb[:, kt, ch * 512:(ch + 1) * 512],
                        start=(kt == 0), stop=False)
            return ps

        def close_group(nt, ps):
            att = attT_t[3][:, nt * 128:(nt + 1) * 128]
            for ch in range(2):
                nc.tensor.matmul(
                    ps[:, ch * 512:(ch + 1) * 512],
                    att,
                    wo_sb[:, 3, ch * 512:(ch + 1) * 512],
                    start=False, stop=True)
            ot = outpool.tile([128, N], f16, tag="out")
            if nt == NT - 1:
                # split the last tile's evac+store so the DMA of the first
                # half overlaps the evacuation of the second (shorter tail)
                for h2 in range(2):
                    sl = slice(h2 * 512, (h2 + 1) * 512)
                    (nc.scalar.activation(ot[:, sl], ps[:, sl], Copy)
                     if h2 == 0 else
                     nc.vector.tensor_copy(ot[:, sl], ps[:, sl]))
                    nc.sync.dma_start(outp[nt * 128:(nt + 1) * 128, sl],
                                      ot[:, sl])
            else:
                if nt % 2 == 0:
                    nc.scalar.activation(ot[:], ps[:], Copy)
                else:
                    nc.vector.tensor_copy(ot[:], ps[:])
                nc.sync.dma_start(outp[nt * 128:(nt + 1) * 128, :], ot[:])

        open_ps = [open_group(0), open_group(1)]
        for nt in range(NT):
            close_group(nt, open_ps[nt])
            if nt + 2 < NT:
                open_ps.append(open_group(nt + 2))

    nc.compile()
    return nc


def get_program(with_qk_bias=True):
    key = ("nc", with_qk_bias)
    if key not in _prog_cache:
        _prog_cache[key] = build_program(with_qk_bias=with_qk_bias)
    return _prog_cache[key]


def make_in_maps(x, attn_bias, attn_mask, Wq, bq, Wk, bk, Wv, bv, Wo, bo):
    """Host-side shard + layout prep.  Returns (in_maps, const) where
    const[c_out] = bo + bv @ Wo.T must be added to the gathered output."""
    x = np.asarray(x, np.float32)
    attn_bias = np.asarray(attn_bias, np.float32)
    attn_mask = np.asarray(attn_mask)
    Wq = np.asarray(Wq, np.float32)
    Wk = np.asarray(Wk, np.float32)
    Wv = np.asarray(Wv, np.float32)
    Wo = np.asarray(Wo, np.float32)
    bq = np.asarray(bq, np.float32)
    bk = np.asarray(bk, np.float32)
    bv = np.asarray(bv, np.float32)
    bo = np.asarray(bo, np.float32)

    const = bo + bv @ Wo.T
    ident = np.eye(128, dtype=np.float16)

    xTs = [np.ascontiguousarray(x[b].T).astype(np.float16) for b in range(B)]
    madds = []
    for b in range(B):
        ma = np.where(attn_mask[b] == 0, np.float32(-1e30), np.float32(0.0))
        madds.append(np.ascontiguousarray(ma.reshape(NT, 128).T))

    in_maps = []
    for core in range(8):
        b, half = divmod(core, 2)
        rows = slice(half * JL, (half + 1) * JL)
        wqT = np.ascontiguousarray(
            (Wq[rows, :] * np.float32(SCALE)).T).astype(np.float16)
        wkT = np.ascontiguousarray(Wk[rows, :].T).astype(np.float16)
        wvT = np.ascontiguousarray(Wv[rows, :].T).astype(np.float16)
        woT = np.ascontiguousarray(Wo[:, rows].T).astype(np.float16)
        bqr = (bq[rows] * np.float32(SCALE)).reshape(1, JL).astype(np.float16)
        bkr = bk[rows].reshape(1, JL).astype(np.float16)
        bT32 = np.ascontiguousarray(
            attn_bias[b, half * HL:(half + 1) * HL].transpose(0, 2, 1))
        # n-half 0 raw (added via identity matmul), n-half 1 exponentiated
        # (folded as exp(S)*exp(B) on DVE)
        bT32[:, :, N // 2:] = np.exp(bT32[:, :, N // 2:])
        bT = bT32.astype(np.float16)
        in_maps.append({
            "xT": xTs[b], "wqT": wqT, "wkT": wkT, "wvT": wvT, "woT": woT,
            "bqr": bqr, "bkr": bkr, "ident": ident, "biasT": bT,
            "madd": madds[b],
        })
    return in_maps, const


def gather(results, const):
    out = np.empty((B, N, C), np.float32)
    for b in range(B):
        out[b] = results[2 * b]["outp"].astype(np.float32) \
            + results[2 * b + 1]["outp"].astype(np.float32) \
            + const[None, :]
    return out


def kernel(**inputs):
    from concourse.bass_utils import run_bass_kernel_spmd
    wb = bool(np.any(np.asarray(inputs["bq"]))
              or np.any(np.asarray(inputs["bk"])))
    nc = get_program(with_qk_bias=wb)
    in_maps, const = make_in_maps(**inputs)
    res = run_bass_kernel_spmd(nc, in_maps, core_ids=list(range(8)))
    return gather(res.results, const)
